# revision 1
# baseline (speedup 1.0000x reference)
"""Tile-parallel 2D Gaussian-splat compositor for Trainium2 (8 NeuronCores).

Strategy
--------
Pixels are sharded across 8 cores as horizontal strips (24 rows each).
Within a core the strip is split into 24x16-pixel tiles (F=384 pixels,
free axis); gaussians go on the partition axis in depth-sorted blocks of
128.  Per (tile, block):

  sigma' = Ghi^T @ feat + Glo^T @ feat   (PE, two f32r passes == exact
                                          fp32: G split into 11+12 mantissa
                                          bit halves, features exact)
  alpha  = exp(-sigma')        (ACT; opacity folded into G's const term)
  am     = alpha * (alpha>=1/255)   (DVE scalar_tensor_tensor, 1 op)
  lg     = ln(1 - am)          (ACT)
  S     += strictU^T @ lg      (PE: cross-partition exclusive cumsum)
  T      = exp(S)              (ACT: per-gaussian transmittance)
  w      = T * am              (DVE)
  rgb   += colors^T @ w        (PE: [3,F] accumulated in PSUM)

Host-side: depth sort, conservative per-gaussian bbox cull per tile
(exact: culled pairs provably have alpha < 1/255 -> zero in the
reference too), quadratic-form coefficients in float64, padding with
inert dummy gaussians so all 8 cores run one SPMD program.

Measured on trn2 (8 cores, steady state via on-device repeat loop):
~33 us per frame composite; rel err vs fp32 reference 2.2e-4.
Key optimizations: single combined exp+ln activation-table set (was 27
table loads -> 1), f32r triangular-cumsum + color matmuls, hi/lo-split
f32r sigma matmul, stage-major wave emission (3 tiles pipelined).
"""

import sys

if "/opt/trn_rl_repo" not in sys.path:
    sys.path.insert(0, "/opt/trn_rl_repo")

import numpy as np

H = 192
W = 192
NDEV = 8
STRIP = H // NDEV            # 24 rows per core
TILE_R = 24                  # tile height == strip height
TILE_C = 16                  # tile width
NT = W // TILE_C             # 12 tiles per core
F = TILE_R * TILE_C          # 384 pixels per tile (matmul free dim)
BLK = 128                    # gaussians per block (partition dim)
ALPHA_MIN = 1.0 / 255.0
ALPHA_MAX = 0.999
DUMMY_SIG = 60.0             # sigma' for padding slots -> alpha ~ 0


def _host_prep(means2d, conics, colors, opacities, depths, background):
    """Sort, cull, and pack per-core parameter arrays (all in float64)."""
    m = np.asarray(means2d, np.float64)
    q = np.asarray(conics, np.float64)
    col = np.asarray(colors, np.float64)
    op = np.asarray(opacities, np.float64)
    dep = np.asarray(depths, np.float64)

    order = np.argsort(dep, kind="stable")
    m = m[order]
    q = q[order]
    col = col[order]
    op = op[order]

    mx, my = m[:, 0], m[:, 1]
    A, B, C = q[:, 0], q[:, 1], q[:, 2]

    with np.errstate(divide="ignore", invalid="ignore"):
        tau = np.log(255.0 * op)
        detq = A * C - B * B
        sxx = C / detq
        syy = A / detq
        ex = np.sqrt(np.maximum(2.0 * tau * sxx, 0.0)) * 1.0001 + 1e-3
        ey = np.sqrt(np.maximum(2.0 * tau * syy, 0.0)) * 1.0001 + 1e-3
    valid = (tau > 0) & (detq > 0) & np.isfinite(ex) & np.isfinite(ey)

    eps = 1e-6
    # gaussian index lists per (device, tile), depth order preserved
    idx = [[None] * NT for _ in range(NDEV)]
    cnt = np.zeros((NDEV, NT), np.int64)
    for d in range(NDEV):
        r0 = d * STRIP
        ymask = valid & (my + ey >= r0 + 0.5 - eps) & (my - ey <= r0 + STRIP - 0.5 + eps)
        for t in range(NT):
            c0 = t * TILE_C
            mask = ymask & (mx + ex >= c0 + 0.5 - eps) & (mx - ex <= c0 + TILE_C - 0.5 + eps)
            g = np.nonzero(mask)[0]
            idx[d][t] = g
            cnt[d, t] = len(g)

    nblk = np.maximum(1, -(-cnt.max(axis=0) // BLK))     # [NT] blocks per tile
    off = np.concatenate([[0], np.cumsum(nblk)])         # [NT+1]
    tot = int(off[-1])

    lnop = np.log(op)
    gts, colss = [], []
    for d in range(NDEV):
        r0 = d * STRIP
        gt = np.zeros((6, tot * BLK), np.float64)
        gt[5, :] = DUMMY_SIG
        cl = np.zeros((BLK, tot * 3), np.float64)
        for t in range(NT):
            g = idx[d][t]
            n = len(g)
            if n == 0:
                continue
            c0 = t * TILE_C
            slot = off[t] * BLK + np.arange(n)
            mlx = mx[g] - (c0 + TILE_C / 2.0)
            mly = my[g] - (r0 + TILE_R / 2.0)
            a, b, c = A[g], B[g], C[g]
            gt[0, slot] = 0.5 * a
            gt[1, slot] = 0.5 * c
            gt[2, slot] = b
            gt[3, slot] = -(a * mlx + b * mly)
            gt[4, slot] = -(c * mly + b * mlx)
            gt[5, slot] = 0.5 * a * mlx**2 + 0.5 * c * mly**2 + b * mlx * mly - lnop[g]
            blk_i = off[t] + np.arange(n) // BLK
            part = np.arange(n) % BLK
            cl[part, blk_i * 3 + 0] = col[g, 0]
            cl[part, blk_i * 3 + 1] = col[g, 1]
            cl[part, blk_i * 3 + 2] = col[g, 2]
        gts.append(gt.astype(np.float32))
        colss.append(cl.astype(np.float32))

    # pixel features in tile-local coords (identical for every tile)
    xs = np.arange(TILE_C) + 0.5 - TILE_C / 2.0
    ys = np.arange(TILE_R) + 0.5 - TILE_R / 2.0
    Y, X = np.meshgrid(ys, xs, indexing="ij")
    x, y = X.ravel(), Y.ravel()
    feat = np.stack([x * x, y * y, x * y, x, y, np.ones(F)]).astype(np.float32)

    strict_u = np.triu(np.ones((BLK, BLK), np.float32), 1)   # [k,n]=1 iff k<n
    compl_u = np.tril(np.ones((BLK, BLK), np.float32), 0)    # [k,n]=1 iff k>=n

    return nblk, off, tot, gts, colss, feat, strict_u, compl_u


def _patch_act_tables():
    """Make Exp and Ln resolve to the single combined activation-table set
    (natural_log_exp_and_others) so the compiler emits ONE table load
    instead of thrashing between exp-only and ln-only sets per op."""
    import functools
    import concourse.bacc as bacc_mod
    import concourse.mybir as mybir
    from concourse.hw_specs import get_activation_tables as orig

    if getattr(bacc_mod.get_activation_tables, "_combined_exp_ln", False):
        return

    @functools.cache
    def patched(arch):
        tabs = {k: set(v) for k, v in orig(arch).items()}
        combined = "natural_log_exp_and_others"
        if combined in tabs:
            Act = mybir.ActivationFunctionType
            for k in tabs:
                if k != combined:
                    tabs[k].discard(Act.Exp)
                    tabs[k].discard(Act.Ln)
        return tabs

    patched._combined_exp_ln = True
    bacc_mod.get_activation_tables = patched


def _build_program(nblk, tot, bg_nonzero, clamp_alpha, f32r_cumsum=True, f32r_color=True,
                   repeat=0, sb_bufs=6, psum_bufs=(3, 3, 2), warmup_mms=0,
                   split_gt_dma=True, split_out_dma=True, window=3, am_on_pool=False,
                   reorder_mask=False, any_copy=True, skew_emission=False):
    import concourse.tile as tile
    import concourse.mybir as mybir
    from concourse import bacc
    from contextlib import ExitStack

    _patch_act_tables()
    f32 = mybir.dt.float32
    f32r = mybir.dt.float32r
    Act = mybir.ActivationFunctionType
    Alu = mybir.AluOpType
    dt_lg = f32r if f32r_cumsum else f32
    dt_w = f32r if f32r_color else f32

    nc = bacc.Bacc("TRN2", target_bir_lowering=False, debug=False)
    feat_d = nc.dram_tensor("feat", [6, F], f32r, kind="ExternalInput")
    ut_d = nc.dram_tensor("ut", [BLK, BLK], dt_lg, kind="ExternalInput")
    gth_d = nc.dram_tensor("gth", [6, tot * BLK], f32r, kind="ExternalInput")
    gtl_d = nc.dram_tensor("gtl", [6, tot * BLK], f32r, kind="ExternalInput")
    cols_d = nc.dram_tensor("cols", [BLK, tot * 3], dt_w, kind="ExternalInput")
    need_compl = bg_nonzero or any(b > 1 for b in nblk)
    if need_compl:
        cu_d = nc.dram_tensor("cu", [BLK, BLK], dt_lg, kind="ExternalInput")
    if bg_nonzero:
        bg_d = nc.dram_tensor("bg", [1, 3], f32, kind="ExternalInput")
    out_d = nc.dram_tensor("out", [3, STRIP, W], f32, kind="ExternalOutput")

    with tile.TileContext(nc) as tc, ExitStack() as ctx:
        cpool = ctx.enter_context(tc.tile_pool(name="consts", bufs=1))
        sb = ctx.enter_context(tc.tile_pool(name="sb", bufs=sb_bufs))
        stp = ctx.enter_context(tc.tile_pool(name="stp", bufs=1))
        ps_sig = ctx.enter_context(tc.tile_pool(name="ps_sig", bufs=psum_bufs[0], space="PSUM"))
        ps_s = ctx.enter_context(tc.tile_pool(name="ps_s", bufs=psum_bufs[1], space="PSUM"))
        ps_col = ctx.enter_context(tc.tile_pool(name="ps_col", bufs=psum_bufs[2], space="PSUM"))

        if warmup_mms:
            # Keep the PE HAM activity window busy while input DMAs land so
            # the first real matmuls run at full clock.
            bf16 = mybir.dt.bfloat16
            ps_warm = ctx.enter_context(tc.tile_pool(name="ps_warm", bufs=1, space="PSUM"))
            wsrc = cpool.tile([BLK, 512], bf16, tag="warm_src")
            nc.gpsimd.memset(wsrc[:], 0)
            wdst = ps_warm.tile([BLK, 512], f32, tag="warm_dst")
            for _ in range(warmup_mms):
                nc.tensor.matmul(wdst[:], wsrc[:, 0:BLK], wsrc[:], start=True, stop=True)

        feat = cpool.tile([6, F], f32r)
        nc.sync.dma_start(feat[:], feat_d.ap())
        ut = cpool.tile([BLK, BLK], dt_lg)
        nc.sync.dma_start(ut[:], ut_d.ap())
        gth = cpool.tile([6, tot * BLK], f32r)
        gtl = cpool.tile([6, tot * BLK], f32r)
        nchunk = 4
        csz = -(-tot // nchunk) * BLK
        for ci in range(nchunk):
            lo_c = ci * csz
            hi_c = min((ci + 1) * csz, tot * BLK)
            if lo_c >= hi_c:
                break
            nc.sync.dma_start(gth[:, lo_c:hi_c], gth_d.ap()[:, lo_c:hi_c])
            nc.sync.dma_start(gtl[:, lo_c:hi_c], gtl_d.ap()[:, lo_c:hi_c])
        gt_tiles = [(gth[:, i * BLK:(i + 1) * BLK], gtl[:, i * BLK:(i + 1) * BLK])
                    for i in range(tot)]
        cols = cpool.tile([BLK, tot * 3], dt_w)
        nc.sync.dma_start(cols[:], cols_d.ap())
        if need_compl:
            cu = cpool.tile([BLK, BLK], dt_lg)
            nc.sync.dma_start(cu[:], cu_d.ap())
        if bg_nonzero:
            bgt = cpool.tile([1, 3], f32)
            nc.sync.dma_start(bgt[:], bg_d.ap())

        out_ap = out_d.ap()

        def body():
            _emit_tiles(nc, tc, mybir, nblk, bg_nonzero, clamp_alpha, need_compl,
                        feat, ut, gt_tiles, cols,
                        cu if need_compl else None,
                        bgt if bg_nonzero else None,
                        sb, stp, ps_sig, ps_s, ps_col, out_ap,
                        f32, dt_lg, dt_w, split_out_dma, window=window,
                        am_on_pool=am_on_pool, reorder_mask=reorder_mask,
                        any_copy=any_copy, skew_emission=skew_emission)

        if repeat:
            with tc.For_i(0, repeat, 1):
                body()
        else:
            body()
    nc.compile()
    return nc


def _emit_tiles(nc, tc, mybir, nblk, bg_nonzero, clamp_alpha, need_compl,
                feat, ut, gt_tiles, cols, cu, bgt,
                sb, stp, ps_sig, ps_s, ps_col, out_ap, f32, dt_lg, dt_w, split_out_dma,
                window=3, am_on_pool=False, reorder_mask=False, any_copy=False,
                skew_emission=False):
    Act = mybir.ActivationFunctionType
    Alu = mybir.AluOpType
    HALF = NT // 2
    strips = []
    for h in range(2):
        sh = stp.tile([3, STRIP * (W // 2)], f32, tag=f"strip{h}", name=f"strip{h}")
        strips.append(sh[:].rearrange("c (h w) -> c h w", h=STRIP))

    # Build one work item per (tile, block); each is a list of stage
    # closures. Emission is stage-major inside a sliding window so every
    # engine always has `window` independent ops queued (better overlap
    # than tile-major emission).
    tiles_state = {}

    def make_block_stages(t, b, bt, blk):
        tst = {}

        def s_sigma():
            if b == 0:
                tiles_state[t] = {
                    "s_ps": ps_s.tile([BLK, F], f32, tag="s_ps", name="s_ps"),
                    "colp": ps_col.tile([3, F], f32, tag="colp", name="colp"),
                    "colbase": 0,
                }
            tst.update(tiles_state[t])
            sig = ps_sig.tile([BLK, F], f32, tag="sig", name="sig")
            tst["sig"] = sig
            nc.tensor.matmul(sig[:], gt_tiles[blk][0], feat[:],
                             start=True, stop=False, skip_group_check=True)
            nc.tensor.matmul(sig[:], gt_tiles[blk][1], feat[:],
                             start=False, stop=True, skip_group_check=True)

        def s_alpha():
            alpha = sb.tile([BLK, F], f32, tag="alpha", name="alpha")
            tst["alpha"] = alpha
            nc.scalar.activation(alpha[:], tst["sig"][:], Act.Exp, scale=-1.0)
            if clamp_alpha:
                nc.vector.tensor_scalar_min(alpha[:], alpha[:], ALPHA_MAX)

        def s_am():
            am = sb.tile([BLK, F], f32, tag="am", name="am")
            tst["am"] = am
            if reorder_mask:
                m2 = sb.tile([BLK, F], f32, tag="m2", name="m2")
                tst["m2"] = m2
                nc.vector.tensor_scalar(m2[:], tst["alpha"][:], ALPHA_MIN, None,
                                        op0=Alu.is_ge)
                nc.vector.tensor_mul(am[:], tst["alpha"][:], m2[:])
            elif am_on_pool:
                m2 = sb.tile([BLK, F], f32, tag="m2", name="m2")
                nc.gpsimd.tensor_scalar(m2[:], tst["alpha"][:], ALPHA_MIN, None,
                                        op0=Alu.is_ge)
                nc.gpsimd.tensor_mul(am[:], m2[:], tst["alpha"][:])
            else:
                nc.vector.scalar_tensor_tensor(am[:], tst["alpha"][:], ALPHA_MIN,
                                               tst["alpha"][:], op0=Alu.is_ge, op1=Alu.mult)

        def s_ln():
            lg = sb.tile([BLK, F], dt_lg, tag="lg", name="lg")
            tst["lg"] = lg
            if reorder_mask:
                lgraw = sb.tile([BLK, F], f32, tag="lgraw", name="lgraw")
                nc.scalar.activation(lgraw[:], tst["alpha"][:], Act.Ln, bias=1.0, scale=-1.0)
                nc.vector.tensor_mul(lg[:], lgraw[:], tst["m2"][:])
            else:
                nc.scalar.activation(lg[:], tst["am"][:], Act.Ln, bias=1.0, scale=-1.0)

        def s_strict():
            nc.tensor.matmul(tst["s_ps"][:], ut[:], tst["lg"][:],
                             start=(b == 0), stop=(b == bt - 1 and not need_compl),
                             skip_group_check=True)

        def s_texp():
            tr = sb.tile([BLK, F], f32, tag="tr", name="tr")
            tst["tr"] = tr
            nc.scalar.activation(tr[:], tst["s_ps"][:], Act.Exp)

        def s_w():
            w = sb.tile([BLK, F], dt_w, tag="w", name="w")
            tst["w"] = w
            nc.vector.tensor_mul(w[:], tst["tr"][:], tst["am"][:])

        def s_color():
            cb = tst["colbase"]
            nc.tensor.matmul(tst["colp"][cb:cb + 3, :],
                             cols[:, blk * 3:(blk + 1) * 3], tst["w"][:],
                             start=(b == 0), stop=(b == bt - 1 and not bg_nonzero),
                             skip_group_check=True)
            if need_compl and (b < bt - 1 or bg_nonzero):
                nc.tensor.matmul(tst["s_ps"][:], cu[:], tst["lg"][:],
                                 start=False, stop=(b == bt - 1), skip_group_check=True)

        def s_out():
            colp = tst["colp"]
            if bg_nonzero:
                tfin = sb.tile([1, F], f32, tag="tfin", name="tfin")
                nc.scalar.activation(tfin[:], tst["s_ps"][0:1, :], Act.Exp)
                nc.tensor.matmul(colp[:], bgt[:], tfin[:],
                                 start=False, stop=True, skip_group_check=True)
            half, tloc = (0, t) if t < HALF else (1, t - HALF)
            copy_eng = nc.any if any_copy else nc.vector
            copy_eng.tensor_copy(
                strips[half][:, :, tloc * TILE_C:(tloc + 1) * TILE_C],
                colp[:].rearrange("c (h w) -> c h w", h=TILE_R))
            if t == HALF - 1:
                nc.sync.dma_start(out_ap[:, :, 0:W // 2], strips[0])
            elif t == NT - 1:
                nc.sync.dma_start(out_ap[:, :, W // 2:W], strips[1])

        st = [s_sigma, s_alpha, s_am, s_ln, s_strict, s_texp, s_w, s_color]
        if b == bt - 1:
            st.append(s_out)
        return st

    stage_lists = []
    for t in range(NT):
        bt = int(nblk[t])
        off_t = int(np.sum(nblk[:t]))
        tile_stages = []
        for b in range(bt):
            tile_stages.extend(make_block_stages(t, b, bt, off_t + b))
        stage_lists.append(tile_stages)

    if skew_emission:
        # Skewed software pipeline: tile i begins `skew` stages after tile
        # i-1, so there is no wave-boundary drain/refill bubble.
        skew = max(1, 9 // window * window // window)  # = 3 for window 3
        skew = window
        nst = [len(s) for s in stage_lists]
        total = skew * (len(stage_lists) - 1) + max(nst)
        for step in range(total):
            for i, st in enumerate(stage_lists):
                s = step - skew * i
                if 0 <= s < len(st):
                    st[s]()
    else:
        i = 0
        while i < len(stage_lists):
            group = stage_lists[i:i + window]
            depth = max(len(s) for s in group)
            for s in range(depth):
                for g in group:
                    if s < len(g):
                        g[s]()
            i += window



def _trunc11(x):
    b = np.ascontiguousarray(np.asarray(x, np.float32)).view(np.uint32)
    return (b & np.uint32(0xFFFFF000)).view(np.float32)


def _make_in_maps(nblk, tot, gts, colss, feat, strict_u, compl_u, bg=None):
    need_compl = (bg is not None) or any(b > 1 for b in nblk)
    maps = []
    for d in range(NDEV):
        hi = _trunc11(gts[d])
        lo = _trunc11(gts[d] - hi)
        im = {"feat": feat, "ut": strict_u, "gth": hi, "gtl": lo,
              "cols": colss[d]}
        if need_compl:
            im["cu"] = compl_u
        if bg is not None:
            im["bg"] = np.asarray(bg, np.float32).reshape(1, 3)
        maps.append(im)
    return maps


def kernel(means2d, conics, colors, opacities, depths, background):
    from concourse import bass_utils

    nblk, off, tot, gts, colss, feat, strict_u, compl_u = _host_prep(
        means2d, conics, colors, opacities, depths, background
    )
    bg = np.asarray(background, np.float32)
    bg_nonzero = bool(np.any(bg != 0))
    clamp_alpha = bool(np.asarray(opacities).max() >= ALPHA_MAX)

    nc = _build_program(nblk, tot, bg_nonzero, clamp_alpha)

    in_maps = _make_in_maps(nblk, tot, gts, colss, feat, strict_u, compl_u,
                            bg if bg_nonzero else None)

    res = bass_utils.run_bass_kernel_spmd(nc, in_maps, core_ids=list(range(NDEV)))
    img = np.concatenate([res.results[d]["out"] for d in range(NDEV)], axis=1)
    return img.astype(np.float32)


if __name__ == "__main__":
    import reference

    inputs = {k: np.asarray(v) for k, v in reference.setup_inputs().items()}
    out = kernel(**inputs)
    print("kernel output:", out.shape, out.dtype)



# revision 13
# speedup vs baseline: 1.2453x; 1.2453x over previous
"""Tile-parallel 2D Gaussian-splat compositor for Trainium2 (8 NeuronCores).

Strategy (v3)
-------------
Pixels sharded across 8 cores as 24-row strips; each strip is 12 tiles of
24x16 px.  Tiles are processed in GROUPS of GW (default 4) adjacent
columns so every ACT/DVE op covers GW*384 pixels (GW PSUM banks),
amortizing the fixed ~185ns SBUF-access cost per instruction; matmuls
stay per-384-px subtile (PSUM bank limit).  Gaussians: depth-sorted,
exact ellipse/rectangle culling per tile, <=127 real per block (row 127
of the last block is always a zero-color dummy).

Per group (one 128-gaussian block per subtile):
  sig   = Ghi^T@feat + Glo^T@feat per subtile (PE, exact fp32 hi/lo split)
  alpha = exp(-sig)                           (ACT, one [128,GW*384] op)
  am    = alpha * (alpha>=1/255)              (DVE scalar_tensor_tensor)
  lg    = ln(1-am)                            (ACT)
  S     = strictU^T @ lg per subtile          (PE, exclusive cumsum)
  T     = exp(S)                              (ACT)
  colp  = dC^T @ T per subtile                (PE, INTO the same work tile,
                                               free after texp read it)
  stg  <- DVE copy colp (PSUM) -> SBUF, interleaved to [3,24,GW*16]
  out  <- one DMA per group

The dC ("delta-color") trick: sum_n c_n*a_n*T_n telescopes to
sum_n (c_n - c_{n-1})*T_n when the last slot's color is zero, removing
the w=T*am multiply entirely; the background term folds into dC as the
color of the first dummy slot.

Gaussian parameter blocks live at PE-quadrant partition offsets
{0,32,64} so all constants arrive in two efficient 128-partition DMAs
(split by partition halves on two DGE queues, overlapped).
"""

import sys

if "/opt/trn_rl_repo" not in sys.path:
    sys.path.insert(0, "/opt/trn_rl_repo")

import numpy as np

H = 192
W = 192
NDEV = 8
STRIP = H // NDEV            # 24 rows per core
TILE_R = 24
TILE_C = 16
NT = W // TILE_C             # 12 tiles per core
F = TILE_R * TILE_C          # 384 px per tile
BLK = 128
ALPHA_MIN = 1.0 / 255.0
ALPHA_MAX = 0.999
DUMMY_SIG = 60.0
GW = 4                       # tiles per group


def _quad_min_over_rect(mx, my, A, B, C, x0, x1, y0, y1):
    """Exact min of 0.5A dx^2 + 0.5C dy^2 + B dx dy over [x0,x1]x[y0,y1],
    vectorized over gaussians (positive-definite quadratic)."""
    inside = (mx >= x0) & (mx <= x1) & (my >= y0) & (my <= y1)
    best = np.full(len(mx), np.inf)

    def f(u, v):
        return 0.5 * A * u * u + 0.5 * C * v * v + B * u * v

    for xe in (x0, x1):
        u = xe - mx
        v = np.clip(-B * u / np.maximum(C, 1e-30), y0 - my, y1 - my)
        best = np.minimum(best, f(u, v))
    for ye in (y0, y1):
        v = ye - my
        u = np.clip(-B * v / np.maximum(A, 1e-30), x0 - mx, x1 - mx)
        best = np.minimum(best, f(u, v))
    return np.where(inside, 0.0, best)


def _host_prep(means2d, conics, colors, opacities, depths, background, gw=GW):
    m = np.asarray(means2d, np.float64)
    q = np.asarray(conics, np.float64)
    col = np.asarray(colors, np.float64)
    op = np.asarray(opacities, np.float64)
    dep = np.asarray(depths, np.float64)
    bg = np.asarray(background, np.float64)

    order = np.argsort(dep, kind="stable")
    m, q, col, op = m[order], q[order], col[order], op[order]
    mx, my = m[:, 0], m[:, 1]
    A, B, C = q[:, 0], q[:, 1], q[:, 2]

    with np.errstate(divide="ignore", invalid="ignore"):
        tau = np.log(255.0 * op)
    valid = (tau > 0) & (A > 0) & (C > 0) & (A * C - B * B > 0)
    lnop = np.where(op > 0, np.log(np.maximum(op, 1e-300)), 0.0)

    # exact per-(device,tile) culling: keep iff min sigma_geo over the
    # tile's pixel-center rectangle <= tau  (else alpha < 1/255 everywhere
    # in the tile, which the reference masks to zero -> exact)
    eps = 1e-9
    idx = [[None] * NT for _ in range(NDEV)]
    cnt = np.zeros((NDEV, NT), np.int64)
    for d in range(NDEV):
        y0, y1 = d * STRIP + 0.5, d * STRIP + STRIP - 0.5
        for t in range(NT):
            x0, x1 = t * TILE_C + 0.5, t * TILE_C + TILE_C - 0.5
            smin = _quad_min_over_rect(mx, my, A, B, C, x0, x1, y0, y1)
            g = np.nonzero(valid & (smin <= tau + eps))[0]
            idx[d][t] = g
            cnt[d, t] = len(g)

    ngr = NT // gw
    # per-tile block count; one slot is reserved for the zero-color dummy
    # that terminates the delta-color telescope
    nblk_t = np.maximum(1, -(-(cnt.max(axis=0) + 1) // BLK))   # [NT]
    gnblk = [int(max(nblk_t[gw * g:gw * (g + 1)])) for g in range(ngr)]
    goff = np.concatenate([[0], np.cumsum(gnblk)]).astype(int)
    nbt = int(goff[-1])          # total group-blocks
    nslot = gw * nbt             # per-subtile gt slots
    nband = -(-nslot // 3)       # 128-col bands of 3 quadrant slots

    gt_slots = np.zeros((NDEV, nslot, 6, BLK), np.float64)
    gt_slots[:, :, 5, :] = DUMMY_SIG
    dc = np.zeros((NDEV, BLK, nslot * 3), np.float64)

    for d in range(NDEV):
        r0 = d * STRIP
        for t in range(NT):
            g = idx[d][t]
            n = len(g)
            gg, sub = t // gw, t % gw
            bt = gnblk[gg]
            assert n <= bt * BLK - 1
            c0 = t * TILE_C
            mlx = mx[g] - (c0 + TILE_C / 2.0)
            mly = my[g] - (r0 + TILE_R / 2.0)
            a, b, c = A[g], B[g], C[g]
            rows = np.arange(n)
            for bi in range(bt):
                s = gw * (goff[gg] + bi) + sub
                sel = (rows // BLK) == bi
                part = rows[sel] % BLK
                gt_slots[d, s, 0, part] = 0.5 * a[sel]
                gt_slots[d, s, 1, part] = 0.5 * c[sel]
                gt_slots[d, s, 2, part] = b[sel]
                gt_slots[d, s, 3, part] = -(a[sel] * mlx[sel] + b[sel] * mly[sel])
                gt_slots[d, s, 4, part] = -(c[sel] * mly[sel] + b[sel] * mlx[sel])
                gt_slots[d, s, 5, part] = (0.5 * a[sel] * mlx[sel] ** 2
                                           + 0.5 * c[sel] * mly[sel] ** 2
                                           + b[sel] * mlx[sel] * mly[sel]
                                           - lnop[g[sel]])
            # delta colors: row k gets c_k - c_{k-1} in depth order across
            # blocks (c_{-1} = 0); dummy slots take the background color so
            # the first dummy row adds bg - c_{n-1} (the bg*T_final term)
            cseq = col[g]
            ext = np.zeros((bt * BLK, 3))
            ext[:n] = cseq
            ext[n:] = bg[None, :]
            dcs = np.diff(np.concatenate([np.zeros((1, 3)), ext], axis=0), axis=0)
            for bi in range(bt):
                s = gw * (goff[gg] + bi) + sub
                dc[d, :, s * 3:(s + 1) * 3] = dcs[bi * BLK:(bi + 1) * BLK]

    # pixel features in tile-local coords, replicated at the 3 PE quadrants
    xs = np.arange(TILE_C) + 0.5 - TILE_C / 2.0
    ys = np.arange(TILE_R) + 0.5 - TILE_R / 2.0
    Y, X = np.meshgrid(ys, xs, indexing="ij")
    x, y = X.ravel(), Y.ravel()
    feat6 = np.stack([x * x, y * y, x * y, x, y, np.ones(F)]).astype(np.float32)
    feat = np.zeros((128, F), np.float32)
    for qd in range(3):
        feat[32 * qd:32 * qd + 6] = feat6

    strict_u = np.triu(np.ones((BLK, BLK), np.float32), 1)
    compl_u = np.tril(np.ones((BLK, BLK), np.float32), 0)
    need_cu = any(b > 1 for b in gnblk)

    def trunc11(xv):
        bb = np.ascontiguousarray(np.asarray(xv, np.float32)).view(np.uint32)
        return (bb & np.uint32(0xFFFFF000)).view(np.float32)

    off_ut = 0
    off_cu = off_ut + BLK
    off_hi = off_cu + (BLK if need_cu else 0)
    off_lo = off_hi + nband * BLK
    off_dc = off_lo + nband * BLK
    off_ft = off_dc + nslot * 3
    XC = off_ft + F
    blobs = []
    for d in range(NDEV):
        blob = np.zeros((128, XC), np.float32)
        blob[:, off_ut:off_ut + BLK] = strict_u
        if need_cu:
            blob[:, off_cu:off_cu + BLK] = compl_u
        g32 = gt_slots[d].astype(np.float32)
        hi = trunc11(g32)
        lo = trunc11(g32 - hi)
        for s in range(nslot):
            p0, cb = 32 * (s % 3), (s // 3) * BLK
            blob[p0:p0 + 6, off_hi + cb:off_hi + cb + BLK] = hi[s]
            blob[p0:p0 + 6, off_lo + cb:off_lo + cb + BLK] = lo[s]
        blob[:, off_dc:off_dc + nslot * 3] = dc[d].astype(np.float32)
        blob[:, off_ft:off_ft + F] = feat
        blobs.append(blob)

    meta = dict(gw=gw, gnblk=gnblk, goff=list(map(int, goff)), nslot=nslot,
                nband=nband, need_cu=need_cu, XC=XC,
                offs=dict(ut=off_ut, cu=off_cu, hi=off_hi, lo=off_lo,
                          dc=off_dc, ft=off_ft),
                clamp_alpha=bool(np.asarray(opacities).max() >= ALPHA_MAX))
    return meta, blobs


def _patch_act_tables():
    """Resolve Exp and Ln to the combined table set so the compiler emits a
    single ACT table load instead of thrashing between per-func sets."""
    import functools
    import concourse.bacc as bacc_mod
    import concourse.mybir as mybir
    from concourse.hw_specs import get_activation_tables as orig

    if getattr(bacc_mod.get_activation_tables, "_combined_exp_ln", False):
        return

    @functools.cache
    def patched(arch):
        tabs = {k: set(v) for k, v in orig(arch).items()}
        combined = "natural_log_exp_and_others"
        if combined in tabs:
            Act = mybir.ActivationFunctionType
            for k in tabs:
                if k != combined:
                    tabs[k].discard(Act.Exp)
                    tabs[k].discard(Act.Ln)
        return tabs

    patched._combined_exp_ln = True
    bacc_mod.get_activation_tables = patched


def _build_program(meta, repeat=0, sb_bufs=4, work_bufs=None, col_bufs=1,
                   window=2, skew=None, split_dma=True):
    import concourse.tile as tile
    import concourse.mybir as mybir
    from concourse import bacc
    from contextlib import ExitStack

    _patch_act_tables()
    f32 = mybir.dt.float32
    f32r = mybir.dt.float32r

    gw = meta["gw"]
    gnblk = meta["gnblk"]
    goff = meta["goff"]
    need_cu = meta["need_cu"]
    offs = meta["offs"]
    XC = meta["XC"]

    if work_bufs is None:
        work_bufs = 8 // gw if not need_cu else (8 - gw) // gw

    nc = bacc.Bacc("TRN2", target_bir_lowering=False, debug=False)
    blob_d = nc.dram_tensor("blob", [128, XC], f32r, kind="ExternalInput")
    out_d = nc.dram_tensor("out", [3, STRIP, W], f32, kind="ExternalOutput")

    with tile.TileContext(nc) as tc, ExitStack() as ctx:
        cpool = ctx.enter_context(tc.tile_pool(name="consts", bufs=1))
        sb = ctx.enter_context(tc.tile_pool(name="sb", bufs=sb_bufs))
        work = ctx.enter_context(tc.tile_pool(name="work", bufs=work_bufs,
                                              space="PSUM"))
        colp_pool = None
        if need_cu:
            colp_pool = ctx.enter_context(tc.tile_pool(name="colp", bufs=col_bufs,
                                                       space="PSUM"))

        cst = cpool.tile([128, XC], f32r)
        if split_dma:
            nc.sync.dma_start(cst[0:64, :], blob_d.ap()[0:64, :])
            nc.gpsimd.dma_start(cst[64:128, :], blob_d.ap()[64:128, :])
        else:
            nc.sync.dma_start(cst[:], blob_d.ap())
        ut = cst[:, offs["ut"]:offs["ut"] + BLK]
        cu = cst[:, offs["cu"]:offs["cu"] + BLK] if need_cu else None
        ft = offs["ft"]

        def gt_ap(kind, s):
            p0, cb = 32 * (s % 3), (s // 3) * BLK
            base = offs[kind] + cb
            return cst[p0:p0 + 6, base:base + BLK]

        def feat_ap(s):
            p0 = 32 * (s % 3)
            return cst[p0:p0 + 6, ft:ft + F]

        def dc_ap(s):
            base = offs["dc"] + s * 3
            return cst[:, base:base + 3]

        out_ap = out_d.ap()

        def body():
            _emit(nc, tc, mybir, gw, gnblk, goff, need_cu, meta["clamp_alpha"],
                  gt_ap, feat_ap, dc_ap, ut, cu, sb, work, colp_pool,
                  out_ap, f32, f32r, window, skew)

        if repeat:
            with tc.For_i(0, repeat, 1):
                body()
        else:
            body()
    nc.compile()
    return nc


def _emit(nc, tc, mybir, gw, gnblk, goff, need_cu, clamp_alpha,
          gt_ap, feat_ap, dc_ap, ut, cu, sb, work, colp_pool,
          out_ap, f32, f32r, window, skew):
    Act = mybir.ActivationFunctionType
    Alu = mybir.AluOpType
    NGR = len(gnblk)
    FW = gw * F                # pixels per group op

    def gview(t):
        # [128, gw, 384] strided view over the gw used bank regions
        return t[:].rearrange("p (b c) -> p b c", b=gw)[:, :, 0:F]

    def make_stages(g):
        st = []
        state = {}
        bt = gnblk[g]

        for b in range(bt):
            last = b == bt - 1

            def s_sigma(b=b):
                sig = work.tile([BLK, 512 * gw], f32, tag="sig", name="sig")
                state["sig"] = sig
                for h in range(gw):
                    s = gw * (goff[g] + b) + h
                    o = 512 * h
                    nc.tensor.matmul(sig[:, o:o + F], gt_ap("hi", s), feat_ap(s),
                                     start=True, stop=False, skip_group_check=True)
                    nc.tensor.matmul(sig[:, o:o + F], gt_ap("lo", s), feat_ap(s),
                                     start=False, stop=True, skip_group_check=True)

            def s_alpha():
                alpha = sb.tile([BLK, FW], f32, tag="alpha", name="alpha")
                state["alpha"] = alpha
                nc.scalar.activation(gview(alpha), gview(state["sig"]),
                                     Act.Exp, scale=-1.0)
                if clamp_alpha:
                    nc.vector.tensor_scalar_min(alpha[:], alpha[:], ALPHA_MAX)

            def s_am():
                am = sb.tile([BLK, FW], f32, tag="am", name="am")
                state["am"] = am
                nc.vector.scalar_tensor_tensor(am[:], state["alpha"][:], ALPHA_MIN,
                                               state["alpha"][:],
                                               op0=Alu.is_ge, op1=Alu.mult)

            def s_ln():
                lg = sb.tile([BLK, FW], f32r, tag="lg", name="lg")
                state["lg"] = lg
                nc.scalar.activation(lg[:], state["am"][:], Act.Ln,
                                     bias=1.0, scale=-1.0)

            def s_strict(b=b, last=last):
                if b == 0:
                    # reuse the sigma PSUM tile: sig is dead after s_alpha and
                    # the first strict matmul start=True resets the banks
                    state["sS"] = state["sig"]
                sS = state["sS"]
                lg = state["lg"]
                for h in range(gw):
                    nc.tensor.matmul(sS[:, 512 * h:512 * h + F], ut,
                                     lg[:, F * h:F * (h + 1)],
                                     start=(b == 0), stop=(last and not need_cu),
                                     skip_group_check=True)

            def s_texp():
                tr = sb.tile([BLK, FW], f32r, tag="tr", name="tr")
                state["tr"] = tr
                nc.scalar.activation(gview(tr), gview(state["sS"]), Act.Exp)

            def s_color(b=b, last=last):
                if b == 0:
                    if need_cu:
                        state["colp"] = colp_pool.tile([3, 512 * gw], f32,
                                                       tag="colp", name="colp")[:]
                    else:
                        # single-block: the work tile is free after texp read
                        # it -> put the color accumulators there (WAR dep)
                        state["colp"] = state["sS"][0:3, :]
                colp = state["colp"]
                tr = state["tr"]
                for h in range(gw):
                    s = gw * (goff[g] + b) + h
                    nc.tensor.matmul(colp[:, 512 * h:512 * h + F], dc_ap(s),
                                     tr[:, F * h:F * (h + 1)],
                                     start=(b == 0), stop=last,
                                     skip_group_check=True)
                if need_cu and not last:
                    sS, lg = state["sS"], state["lg"]
                    for h in range(gw):
                        nc.tensor.matmul(sS[:, 512 * h:512 * h + F], cu,
                                         lg[:, F * h:F * (h + 1)],
                                         start=False, stop=(b == bt - 2),
                                         skip_group_check=True)

            def s_out(g=g):
                colp = state["colp"]
                stg = sb.tile([3, FW], f32, tag="stg", name="stg")
                # interleave the gw 24x16 subtiles into [3,24,gw*16] so the
                # out-DMA is a plain 3D transfer (DVE: gpsimd can't read PSUM)
                nc.vector.tensor_copy(
                    stg[:].rearrange("c (h b w) -> c b h w", b=gw, w=TILE_C),
                    colp.rearrange("c (b x) -> c b x", b=gw)[:, :, 0:F]
                        .rearrange("c b (h w) -> c b h w", h=TILE_R))
                src = stg[:].rearrange("c (h w) -> c h w", h=TILE_R)
                dst = out_ap[:, :, gw * g * TILE_C:gw * (g + 1) * TILE_C]
                nc.gpsimd.dma_start(dst, src)

            st.extend([s_sigma, s_alpha, s_am, s_ln, s_strict, s_texp, s_color])
            if last:
                st.append(s_out)
        return st

    stage_lists = [make_stages(g) for g in range(NGR)]

    if skew is not None:
        nst = [len(s) for s in stage_lists]
        total = skew * (NGR - 1) + max(nst)
        for step in range(total):
            for i, st in enumerate(stage_lists):
                s = step - skew * i
                if 0 <= s < len(st):
                    st[s]()
    else:
        i = 0
        while i < len(stage_lists):
            group = stage_lists[i:i + window]
            depth = max(len(s) for s in group)
            for s in range(depth):
                for gl in group:
                    if s < len(gl):
                        gl[s]()
            i += window


def _make_in_maps(blobs):
    return [{"blob": blobs[d]} for d in range(NDEV)]


def kernel(means2d, conics, colors, opacities, depths, background):
    from concourse import bass_utils

    meta, blobs = _host_prep(means2d, conics, colors, opacities, depths,
                             background)
    nc = _build_program(meta)
    in_maps = _make_in_maps(blobs)
    res = bass_utils.run_bass_kernel_spmd(nc, in_maps, core_ids=list(range(NDEV)))
    img = np.concatenate([res.results[d]["out"] for d in range(NDEV)], axis=1)
    return img.astype(np.float32)


if __name__ == "__main__":
    import reference

    inputs = {k: np.asarray(v) for k, v in reference.setup_inputs().items()}
    out = kernel(**inputs)
    print("kernel output:", out.shape, out.dtype)


# revision 32
# speedup vs baseline: 1.4250x; 1.1443x over previous
"""Tile-parallel 2D Gaussian-splat compositor for Trainium2 (8 NeuronCores).

Strategy (v3)
-------------
Pixels sharded across 8 cores as 24-row strips; each strip is 12 tiles of
24x16 px.  Tiles are processed in GROUPS of GW (default 4) adjacent
columns so every ACT/DVE op covers GW*384 pixels (GW PSUM banks),
amortizing the fixed ~185ns SBUF-access cost per instruction; matmuls
stay per-384-px subtile (PSUM bank limit).  Gaussians: depth-sorted,
exact ellipse/rectangle culling per tile, <=127 real per block (row 127
of the last block is always a zero-color dummy).

Per group (one 128-gaussian block per subtile):
  sig   = Ghi^T@feat + Glo^T@feat per subtile (PE, exact fp32 hi/lo split)
  alpha = exp(-sig)                           (ACT, one [128,GW*384] op)
  am    = alpha * (alpha>=1/255)              (DVE scalar_tensor_tensor)
  lg    = ln(1-am)                            (ACT)
  S     = strictU^T @ lg per subtile          (PE, exclusive cumsum)
  T     = exp(S)                              (ACT)
  colp  = dC^T @ T per subtile                (PE, INTO the same work tile,
                                               free after texp read it)
  stg  <- DVE copy colp (PSUM) -> SBUF, interleaved to [3,24,GW*16]
  out  <- one DMA per group

The dC ("delta-color") trick: sum_n c_n*a_n*T_n telescopes to
sum_n (c_n - c_{n-1})*T_n when the last slot's color is zero, removing
the w=T*am multiply entirely; the background term folds into dC as the
color of the first dummy slot.

Gaussian parameter blocks live at PE-quadrant partition offsets
{0,32,64} so all constants arrive in two efficient 128-partition DMAs
(split by partition halves on two DGE queues, overlapped).
"""

import sys

if "/opt/trn_rl_repo" not in sys.path:
    sys.path.insert(0, "/opt/trn_rl_repo")

import numpy as np

H = 192
W = 192
NDEV = 8
STRIP = H // NDEV            # 24 rows per core
TILE_R = 24
TILE_C = 16
NT = W // TILE_C             # 12 tiles per core
F = TILE_R * TILE_C          # 384 px per tile
BLK = 128
ALPHA_MIN = 1.0 / 255.0
ALPHA_MAX = 0.999
DUMMY_SIG = 60.0
GW = 2                       # tiles per group


def _quad_min_over_rect(mx, my, A, B, C, x0, x1, y0, y1):
    """Exact min of 0.5A dx^2 + 0.5C dy^2 + B dx dy over [x0,x1]x[y0,y1],
    vectorized over gaussians (positive-definite quadratic)."""
    inside = (mx >= x0) & (mx <= x1) & (my >= y0) & (my <= y1)
    best = np.full(len(mx), np.inf)

    def f(u, v):
        return 0.5 * A * u * u + 0.5 * C * v * v + B * u * v

    for xe in (x0, x1):
        u = xe - mx
        v = np.clip(-B * u / np.maximum(C, 1e-30), y0 - my, y1 - my)
        best = np.minimum(best, f(u, v))
    for ye in (y0, y1):
        v = ye - my
        u = np.clip(-B * v / np.maximum(A, 1e-30), x0 - mx, x1 - mx)
        best = np.minimum(best, f(u, v))
    return np.where(inside, 0.0, best)


def _host_prep(means2d, conics, colors, opacities, depths, background, gw=GW):
    m = np.asarray(means2d, np.float64)
    q = np.asarray(conics, np.float64)
    col = np.asarray(colors, np.float64)
    op = np.asarray(opacities, np.float64)
    dep = np.asarray(depths, np.float64)
    bg = np.asarray(background, np.float64)

    order = np.argsort(dep, kind="stable")
    m, q, col, op = m[order], q[order], col[order], op[order]
    mx, my = m[:, 0], m[:, 1]
    A, B, C = q[:, 0], q[:, 1], q[:, 2]

    with np.errstate(divide="ignore", invalid="ignore"):
        tau = np.log(255.0 * op)
    valid = (tau > 0) & (A > 0) & (C > 0) & (A * C - B * B > 0)
    lnop = np.where(op > 0, np.log(np.maximum(op, 1e-300)), 0.0)

    # exact per-(device,tile) culling: keep iff min sigma_geo over the
    # tile's pixel-center rectangle <= tau  (else alpha < 1/255 everywhere
    # in the tile, which the reference masks to zero -> exact)
    eps = 1e-9
    idx = [[None] * NT for _ in range(NDEV)]
    cnt = np.zeros((NDEV, NT), np.int64)
    for d in range(NDEV):
        y0, y1 = d * STRIP + 0.5, d * STRIP + STRIP - 0.5
        for t in range(NT):
            x0, x1 = t * TILE_C + 0.5, t * TILE_C + TILE_C - 0.5
            smin = _quad_min_over_rect(mx, my, A, B, C, x0, x1, y0, y1)
            g = np.nonzero(valid & (smin <= tau + eps))[0]
            idx[d][t] = g
            cnt[d, t] = len(g)

    ngr = NT // gw
    # per-tile block count; one slot is reserved for the zero-color dummy
    # that terminates the delta-color telescope
    nblk_t = np.maximum(1, -(-(cnt.max(axis=0) + 1) // BLK))   # [NT]
    gnblk = [int(max(nblk_t[gw * g:gw * (g + 1)])) for g in range(ngr)]
    goff = np.concatenate([[0], np.cumsum(gnblk)]).astype(int)
    nbt = int(goff[-1])          # total group-blocks
    nslot = gw * nbt             # per-subtile gt slots
    nband = -(-nslot // 3)       # 128-col bands of 3 quadrant slots

    gt_slots = np.zeros((NDEV, nslot, 6, BLK), np.float64)
    gt_slots[:, :, 5, :] = DUMMY_SIG
    dc = np.zeros((NDEV, BLK, nslot * 3), np.float64)

    for d in range(NDEV):
        r0 = d * STRIP
        for t in range(NT):
            g = idx[d][t]
            n = len(g)
            gg, sub = t // gw, t % gw
            bt = gnblk[gg]
            assert n <= bt * BLK - 1
            c0 = t * TILE_C
            mlx = mx[g] - (c0 + TILE_C / 2.0)
            mly = my[g] - (r0 + TILE_R / 2.0)
            a, b, c = A[g], B[g], C[g]
            rows = np.arange(n)
            for bi in range(bt):
                s = gw * (goff[gg] + bi) + sub
                sel = (rows // BLK) == bi
                part = rows[sel] % BLK
                gt_slots[d, s, 0, part] = 0.5 * a[sel]
                gt_slots[d, s, 1, part] = 0.5 * c[sel]
                gt_slots[d, s, 2, part] = b[sel]
                gt_slots[d, s, 3, part] = -(a[sel] * mlx[sel] + b[sel] * mly[sel])
                gt_slots[d, s, 4, part] = -(c[sel] * mly[sel] + b[sel] * mlx[sel])
                gt_slots[d, s, 5, part] = (0.5 * a[sel] * mlx[sel] ** 2
                                           + 0.5 * c[sel] * mly[sel] ** 2
                                           + b[sel] * mlx[sel] * mly[sel]
                                           - lnop[g[sel]])
            # delta colors: row k gets c_k - c_{k-1} in depth order across
            # blocks (c_{-1} = 0); dummy slots take the background color so
            # the first dummy row adds bg - c_{n-1} (the bg*T_final term)
            cseq = col[g]
            ext = np.zeros((bt * BLK, 3))
            ext[:n] = cseq
            ext[n:] = bg[None, :]
            dcs = np.diff(np.concatenate([np.zeros((1, 3)), ext], axis=0), axis=0)
            for bi in range(bt):
                s = gw * (goff[gg] + bi) + sub
                dc[d, :, s * 3:(s + 1) * 3] = dcs[bi * BLK:(bi + 1) * BLK]

    # pixel features in tile-local coords, replicated at the 3 PE quadrants
    xs = np.arange(TILE_C) + 0.5 - TILE_C / 2.0
    ys = np.arange(TILE_R) + 0.5 - TILE_R / 2.0
    Y, X = np.meshgrid(ys, xs, indexing="ij")
    x, y = X.ravel(), Y.ravel()
    feat6 = np.stack([x * x, y * y, x * y, x, y, np.ones(F)]).astype(np.float32)
    feat = np.zeros((128, F), np.float32)
    for qd in range(3):
        feat[32 * qd:32 * qd + 6] = feat6

    strict_u = np.triu(np.ones((BLK, BLK), np.float32), 1)
    compl_u = np.tril(np.ones((BLK, BLK), np.float32), 0)
    need_cu = any(b > 1 for b in gnblk)

    def trunc11(xv):
        # round-to-nearest at 11 explicit mantissa bits (the f32r PE input
        # precision); hi/lo stays exact and single-pass error is unbiased
        bb = np.ascontiguousarray(np.asarray(xv, np.float32)).view(np.uint32)
        return ((bb + np.uint32(0x800)) & np.uint32(0xFFFFF000)).view(np.float32)

    off_ut = 0
    off_cu = off_ut + BLK
    off_hi = off_cu + (BLK if need_cu else 0)
    off_lo = off_hi + nband * BLK
    off_dc = off_lo + nband * BLK
    off_ft = off_dc + nslot * 3
    XC = off_ft + F
    blobs = []
    for d in range(NDEV):
        blob = np.zeros((128, XC), np.float32)
        blob[:, off_ut:off_ut + BLK] = strict_u
        if need_cu:
            blob[:, off_cu:off_cu + BLK] = compl_u
        g32 = gt_slots[d].astype(np.float32)
        hi = trunc11(g32)
        lo = trunc11(g32 - hi)
        for s in range(nslot):
            p0, cb = 32 * (s % 3), (s // 3) * BLK
            blob[p0:p0 + 6, off_hi + cb:off_hi + cb + BLK] = hi[s]
            blob[p0:p0 + 6, off_lo + cb:off_lo + cb + BLK] = lo[s]
        blob[:, off_dc:off_dc + nslot * 3] = dc[d].astype(np.float32)
        blob[:, off_ft:off_ft + F] = feat
        blobs.append(blob)

    meta = dict(gw=gw, gnblk=gnblk, goff=list(map(int, goff)), nslot=nslot,
                nband=nband, need_cu=need_cu, XC=XC,
                offs=dict(ut=off_ut, cu=off_cu, hi=off_hi, lo=off_lo,
                          dc=off_dc, ft=off_ft),
                clamp_alpha=bool(np.asarray(opacities).max() >= ALPHA_MAX))
    return meta, blobs


def _patch_act_tables():
    """Resolve Exp and Ln to the combined table set so the compiler emits a
    single ACT table load instead of thrashing between per-func sets."""
    import functools
    import concourse.bacc as bacc_mod
    import concourse.mybir as mybir
    from concourse.hw_specs import get_activation_tables as orig

    if getattr(bacc_mod.get_activation_tables, "_combined_exp_ln", False):
        return

    @functools.cache
    def patched(arch):
        tabs = {k: set(v) for k, v in orig(arch).items()}
        combined = "natural_log_exp_and_others"
        if combined in tabs:
            Act = mybir.ActivationFunctionType
            for k in tabs:
                if k != combined:
                    tabs[k].discard(Act.Exp)
                    tabs[k].discard(Act.Ln)
        return tabs

    patched._combined_exp_ln = True
    bacc_mod.get_activation_tables = patched


def _build_program(meta, repeat=0, sb_bufs=4, work_bufs=None, col_bufs=1,
                   window=3, skew=None, split_dma=True, warmup_mms=10,
                   tail_mode=1, hilo=0):
    import concourse.tile as tile
    import concourse.mybir as mybir
    from concourse import bacc
    from contextlib import ExitStack

    _patch_act_tables()
    f32 = mybir.dt.float32
    f32r = mybir.dt.float32r

    gw = meta["gw"]
    gnblk = meta["gnblk"]
    goff = meta["goff"]
    need_cu = meta["need_cu"]
    offs = meta["offs"]
    XC = meta["XC"]

    if work_bufs is None:
        work_bufs = 8 // gw if not need_cu else (8 - gw) // gw

    nc = bacc.Bacc("TRN2", target_bir_lowering=False, debug=False)
    blob_d = nc.dram_tensor("blob", [128, XC], f32r, kind="ExternalInput")
    out_d = nc.dram_tensor("out", [3, STRIP, W], f32, kind="ExternalOutput")

    with tile.TileContext(nc) as tc, ExitStack() as ctx:
        cpool = ctx.enter_context(tc.tile_pool(name="consts", bufs=1))
        sb = ctx.enter_context(tc.tile_pool(name="sb", bufs=sb_bufs))
        work = ctx.enter_context(tc.tile_pool(name="work", bufs=work_bufs,
                                              space="PSUM"))
        colp_pool = None
        if need_cu:
            colp_pool = ctx.enter_context(tc.tile_pool(name="colp", bufs=col_bufs,
                                                       space="PSUM"))

        if warmup_mms:
            # Ramp the PE pstate while the input DMA lands. Uses the work
            # pool's own rotation, so no extra PSUM banks.
            bf16 = mybir.dt.bfloat16
            wsrc = cpool.tile([BLK, 512], bf16, tag="warm_src")
            nc.gpsimd.memset(wsrc[:], 0)
            wdst = work.tile([BLK, 512 * gw], f32, tag="sig", name="warm")
            for _ in range(warmup_mms):
                nc.tensor.matmul(wdst[:, 0:512], wsrc[:, 0:BLK], wsrc[:],
                                 start=True, stop=True, skip_group_check=True)

        cst = cpool.tile([128, XC], f32r)
        if split_dma:
            nc.sync.dma_start(cst[0:64, :], blob_d.ap()[0:64, :])
            nc.gpsimd.dma_start(cst[64:128, :], blob_d.ap()[64:128, :])
        else:
            nc.sync.dma_start(cst[:], blob_d.ap())
        ut = cst[:, offs["ut"]:offs["ut"] + BLK]
        cu = cst[:, offs["cu"]:offs["cu"] + BLK] if need_cu else None
        ft = offs["ft"]

        def gt_ap(kind, s):
            p0, cb = 32 * (s % 3), (s // 3) * BLK
            base = offs[kind] + cb
            return cst[p0:p0 + 6, base:base + BLK]

        def feat_ap(s):
            p0 = 32 * (s % 3)
            return cst[p0:p0 + 6, ft:ft + F]

        def dc_ap(s):
            base = offs["dc"] + s * 3
            return cst[:, base:base + 3]

        out_ap = out_d.ap()

        def body():
            _emit(nc, tc, mybir, gw, gnblk, goff, need_cu, meta["clamp_alpha"],
                  gt_ap, feat_ap, dc_ap, ut, cu, sb, work, colp_pool,
                  out_ap, f32, f32r, window, skew, tail_mode=tail_mode,
                  hilo=hilo)

        if repeat:
            with tc.For_i(0, repeat, 1):
                body()
        else:
            body()
    nc.compile()
    return nc


def _emit(nc, tc, mybir, gw, gnblk, goff, need_cu, clamp_alpha,
          gt_ap, feat_ap, dc_ap, ut, cu, sb, work, colp_pool,
          out_ap, f32, f32r, window, skew, tail_mode=0, hilo=0):
    Act = mybir.ActivationFunctionType
    Alu = mybir.AluOpType
    NGR = len(gnblk)
    FW = gw * F                # pixels per group op

    def gview(t):
        # [128, gw, 384] strided view over the gw used bank regions
        return t[:].rearrange("p (b c) -> p b c", b=gw)[:, :, 0:F]

    def make_stages(g):
        st = []
        state = {}
        bt = gnblk[g]

        for b in range(bt):
            last = b == bt - 1

            def s_sigma(b=b):
                sig = work.tile([BLK, 512 * gw], f32, tag="sig", name="sig")
                state["sig"] = sig
                for h in range(gw):
                    s = gw * (goff[g] + b) + h
                    o = 512 * h
                    if hilo:
                        nc.tensor.matmul(sig[:, o:o + F], gt_ap("hi", s),
                                         feat_ap(s), start=True, stop=False,
                                         skip_group_check=True)
                        nc.tensor.matmul(sig[:, o:o + F], gt_ap("lo", s),
                                         feat_ap(s), start=False, stop=True,
                                         skip_group_check=True)
                    else:
                        nc.tensor.matmul(sig[:, o:o + F], gt_ap("hi", s),
                                         feat_ap(s), start=True, stop=True,
                                         skip_group_check=True)

            def s_alpha():
                alpha = sb.tile([BLK, FW], f32, tag="alpha", name="alpha")
                state["alpha"] = alpha
                nc.scalar.activation(gview(alpha), gview(state["sig"]),
                                     Act.Exp, scale=-1.0)
                if clamp_alpha:
                    nc.vector.tensor_scalar_min(alpha[:], alpha[:], ALPHA_MAX)

            def s_ln():
                # unmasked ln(1-alpha): back-to-back with s_alpha on the ACT
                # queue (no DVE round-trip on the critical chain)
                lgr = sb.tile([BLK, FW], f32, tag="lgr", name="lgr")
                state["lgr"] = lgr
                nc.scalar.activation(lgr[:], state["alpha"][:], Act.Ln,
                                     bias=1.0, scale=-1.0)

            def s_am():
                # lg = (alpha >= 1/255) * ln(1-alpha), one DVE op; ln and
                # alpha stay back-to-back on the ACT queue
                lg = sb.tile([BLK, FW], f32r, tag="lg", name="lg")
                state["lg"] = lg
                nc.vector.scalar_tensor_tensor(lg[:], state["alpha"][:],
                                               ALPHA_MIN, state["lgr"][:],
                                               op0=Alu.is_ge, op1=Alu.mult)

            def s_strict(b=b, last=last):
                if b == 0:
                    # reuse the sigma PSUM tile: sig is dead after s_alpha and
                    # the first strict matmul start=True resets the banks
                    state["sS"] = state["sig"]
                sS = state["sS"]
                lg = state["lg"]
                for h in range(gw):
                    nc.tensor.matmul(sS[:, 512 * h:512 * h + F], ut,
                                     lg[:, F * h:F * (h + 1)],
                                     start=(b == 0), stop=(last and not need_cu),
                                     skip_group_check=True)

            def s_texp():
                tr = sb.tile([BLK, FW], f32r, tag="tr", name="tr")
                state["tr"] = tr
                nc.scalar.activation(gview(tr), gview(state["sS"]), Act.Exp)

            def s_color(b=b, last=last):
                if b == 0:
                    if need_cu:
                        state["colp"] = colp_pool.tile([3, 512 * gw], f32,
                                                       tag="colp", name="colp")[:]
                    else:
                        # single-block: the work tile is free after texp read
                        # it -> put the color accumulators there (WAR dep)
                        state["colp"] = state["sS"][0:3, :]
                colp = state["colp"]
                tr = state["tr"]
                for h in range(gw):
                    s = gw * (goff[g] + b) + h
                    nc.tensor.matmul(colp[:, 512 * h:512 * h + F], dc_ap(s),
                                     tr[:, F * h:F * (h + 1)],
                                     start=(b == 0), stop=last,
                                     skip_group_check=True)
                if need_cu and not last:
                    sS, lg = state["sS"], state["lg"]
                    for h in range(gw):
                        nc.tensor.matmul(sS[:, 512 * h:512 * h + F], cu,
                                         lg[:, F * h:F * (h + 1)],
                                         start=False, stop=(b == bt - 2),
                                         skip_group_check=True)

            def s_out(g=g):
                colp = state["colp"]
                if g == NGR - 1 and tail_mode:
                    # drain tail: split the copy/DMA chain per subtile so the
                    # serial copy->dma->sem chain after the last texp shrinks
                    for h in range(gw):
                        stg = sb.tile([3, F], f32, tag="stg", name="stg")
                        srcv = colp[:, 512 * h:512 * h + F]
                        if tail_mode == 2 and h % 2 == 1:
                            nc.scalar.copy(stg[:], srcv)
                        else:
                            nc.vector.tensor_copy(stg[:], srcv)
                        t = gw * g + h
                        dst = out_ap[:, :, t * TILE_C:(t + 1) * TILE_C]
                        src = stg[:].rearrange("c (h w) -> c h w", h=TILE_R)
                        if h % 2 == 0:
                            nc.gpsimd.dma_start(dst, src)
                        else:
                            nc.sync.dma_start(dst, src)
                    return
                stg = sb.tile([3, FW], f32, tag="stgw", name="stgw")
                # interleave the gw 24x16 subtiles into [3,24,gw*16] so the
                # out-DMA is a plain 3D transfer (DVE: gpsimd can't read PSUM)
                nc.vector.tensor_copy(
                    stg[:].rearrange("c (h b w) -> c b h w", b=gw, w=TILE_C),
                    colp.rearrange("c (b x) -> c b x", b=gw)[:, :, 0:F]
                        .rearrange("c b (h w) -> c b h w", h=TILE_R))
                src = stg[:].rearrange("c (h w) -> c h w", h=TILE_R)
                dst = out_ap[:, :, gw * g * TILE_C:gw * (g + 1) * TILE_C]
                nc.gpsimd.dma_start(dst, src)

            st.extend([s_sigma, s_alpha, s_ln, s_am, s_strict, s_texp,
                       s_color])
            if last:
                st.append(s_out)
        return st

    stage_lists = [make_stages(g) for g in range(NGR)]

    if skew is not None:
        nst = [len(s) for s in stage_lists]
        total = skew * (NGR - 1) + max(nst)
        for step in range(total):
            for i, st in enumerate(stage_lists):
                s = step - skew * i
                if 0 <= s < len(st):
                    st[s]()
    else:
        i = 0
        while i < len(stage_lists):
            group = stage_lists[i:i + window]
            depth = max(len(s) for s in group)
            for s in range(depth):
                for gl in group:
                    if s < len(gl):
                        gl[s]()
            i += window


def _make_in_maps(blobs):
    return [{"blob": blobs[d]} for d in range(NDEV)]


def kernel(means2d, conics, colors, opacities, depths, background):
    from concourse import bass_utils

    meta, blobs = _host_prep(means2d, conics, colors, opacities, depths,
                             background)
    nc = _build_program(meta)
    in_maps = _make_in_maps(blobs)
    res = bass_utils.run_bass_kernel_spmd(nc, in_maps, core_ids=list(range(NDEV)))
    img = np.concatenate([res.results[d]["out"] for d in range(NDEV)], axis=1)
    return img.astype(np.float32)


if __name__ == "__main__":
    import reference

    inputs = {k: np.asarray(v) for k, v in reference.setup_inputs().items()}
    out = kernel(**inputs)
    print("kernel output:", out.shape, out.dtype)


# revision 42
# speedup vs baseline: 1.4351x; 1.0071x over previous
"""Tile-parallel 2D Gaussian-splat compositor for Trainium2 (8 NeuronCores).

Strategy (v4)
-------------
Pixels sharded across 8 cores as 24-row strips; each strip is 12 tiles of
24x16 px.  Tiles are processed in GROUPS of GW adjacent columns so every
ACT/DVE op covers GW*384 pixels (GW PSUM banks), amortizing the ~400ns
fixed cost per ACT instruction measured on HW; matmuls stay per-384-px
subtile (PSUM bank limit).  Gaussians: depth-sorted, exact
ellipse/rectangle culling per tile (quadratic minimization over the
tile rect), <=127 real per block (row 127 of the last block is always a
zero-color dummy).

Per group (one 128-gaussian block per subtile):
  sig   = G^T @ feat per subtile     (PE f32r; rn-11-bit coeffs; optional
                                      exact hi/lo split via hilo=1)
  alpha = exp(-sig)                  (ACT, one [128,GW*384] op)
  lgr   = ln(1-alpha)                (ACT, back-to-back with alpha)
  lg    = (alpha>=1/255)*lgr         (DVE scalar_tensor_tensor)
  S     = strictU^T @ lg per subtile (PE, exclusive cumsum across depth)
  T     = exp(S)                     (ACT)
  colp  = dC^T @ T per subtile      (PE -> per-subtile PSUM bank, or into
                                      the freed work tile when colp_own=0)
  out  <- DVE copy PSUM->SBUF, then DMA per subtile/group

The dC ("delta-color") trick: sum_n c_n*a_n*T_n telescopes to
sum_n (c_n - c_{n-1})*T_n when the last slot's color is zero, removing
the w=T*am multiply entirely; the background term folds into dC as the
color of the first dummy slot.

Gaussian parameter blocks live at PE-quadrant partition offsets
{0,32,64} so all constants arrive in two efficient 128-partition DMAs
(split by partition halves on two DGE queues, overlapped).
"""

import sys

if "/opt/trn_rl_repo" not in sys.path:
    sys.path.insert(0, "/opt/trn_rl_repo")

import numpy as np

H = 192
W = 192
NDEV = 8
STRIP = H // NDEV            # 24 rows per core
TILE_R = 24
TILE_C = 16
NT = W // TILE_C             # 12 tiles per core
F = TILE_R * TILE_C          # 384 px per tile
BLK = 128
ALPHA_MIN = 1.0 / 255.0
ALPHA_MAX = 0.999
DUMMY_SIG = 60.0
GW = 2                       # tiles per group


def _quad_min_over_rect(mx, my, A, B, C, x0, x1, y0, y1):
    """Exact min of 0.5A dx^2 + 0.5C dy^2 + B dx dy over [x0,x1]x[y0,y1],
    vectorized over gaussians (positive-definite quadratic)."""
    inside = (mx >= x0) & (mx <= x1) & (my >= y0) & (my <= y1)
    best = np.full(len(mx), np.inf)

    def f(u, v):
        return 0.5 * A * u * u + 0.5 * C * v * v + B * u * v

    for xe in (x0, x1):
        u = xe - mx
        v = np.clip(-B * u / np.maximum(C, 1e-30), y0 - my, y1 - my)
        best = np.minimum(best, f(u, v))
    for ye in (y0, y1):
        v = ye - my
        u = np.clip(-B * v / np.maximum(A, 1e-30), x0 - mx, x1 - mx)
        best = np.minimum(best, f(u, v))
    return np.where(inside, 0.0, best)


def _host_prep(means2d, conics, colors, opacities, depths, background, gw=GW):
    m = np.asarray(means2d, np.float64)
    q = np.asarray(conics, np.float64)
    col = np.asarray(colors, np.float64)
    op = np.asarray(opacities, np.float64)
    dep = np.asarray(depths, np.float64)
    bg = np.asarray(background, np.float64)

    order = np.argsort(dep, kind="stable")
    m, q, col, op = m[order], q[order], col[order], op[order]
    mx, my = m[:, 0], m[:, 1]
    A, B, C = q[:, 0], q[:, 1], q[:, 2]

    with np.errstate(divide="ignore", invalid="ignore"):
        tau = np.log(255.0 * op)
    valid = (tau > 0) & (A > 0) & (C > 0) & (A * C - B * B > 0)
    lnop = np.where(op > 0, np.log(np.maximum(op, 1e-300)), 0.0)

    # exact per-(device,tile) culling: keep iff min sigma_geo over the
    # tile's pixel-center rectangle <= tau  (else alpha < 1/255 everywhere
    # in the tile, which the reference masks to zero -> exact)
    eps = 1e-9
    idx = [[None] * NT for _ in range(NDEV)]
    cnt = np.zeros((NDEV, NT), np.int64)
    for d in range(NDEV):
        y0, y1 = d * STRIP + 0.5, d * STRIP + STRIP - 0.5
        for t in range(NT):
            x0, x1 = t * TILE_C + 0.5, t * TILE_C + TILE_C - 0.5
            smin = _quad_min_over_rect(mx, my, A, B, C, x0, x1, y0, y1)
            g = np.nonzero(valid & (smin <= tau + eps))[0]
            idx[d][t] = g
            cnt[d, t] = len(g)

    ngr = NT // gw
    # per-tile block count; one slot is reserved for the zero-color dummy
    # that terminates the delta-color telescope
    nblk_t = np.maximum(1, -(-(cnt.max(axis=0) + 1) // BLK))   # [NT]
    gnblk = [int(max(nblk_t[gw * g:gw * (g + 1)])) for g in range(ngr)]
    goff = np.concatenate([[0], np.cumsum(gnblk)]).astype(int)
    nbt = int(goff[-1])          # total group-blocks
    nslot = gw * nbt             # per-subtile gt slots
    nband = -(-nslot // 3)       # 128-col bands of 3 quadrant slots

    gt_slots = np.zeros((NDEV, nslot, 6, BLK), np.float64)
    gt_slots[:, :, 5, :] = DUMMY_SIG
    dc = np.zeros((NDEV, BLK, nslot * 3), np.float64)

    for d in range(NDEV):
        r0 = d * STRIP
        for t in range(NT):
            g = idx[d][t]
            n = len(g)
            gg, sub = t // gw, t % gw
            bt = gnblk[gg]
            assert n <= bt * BLK - 1
            c0 = t * TILE_C
            mlx = mx[g] - (c0 + TILE_C / 2.0)
            mly = my[g] - (r0 + TILE_R / 2.0)
            a, b, c = A[g], B[g], C[g]
            rows = np.arange(n)
            for bi in range(bt):
                s = gw * (goff[gg] + bi) + sub
                sel = (rows // BLK) == bi
                part = rows[sel] % BLK
                gt_slots[d, s, 0, part] = 0.5 * a[sel]
                gt_slots[d, s, 1, part] = 0.5 * c[sel]
                gt_slots[d, s, 2, part] = b[sel]
                gt_slots[d, s, 3, part] = -(a[sel] * mlx[sel] + b[sel] * mly[sel])
                gt_slots[d, s, 4, part] = -(c[sel] * mly[sel] + b[sel] * mlx[sel])
                gt_slots[d, s, 5, part] = (0.5 * a[sel] * mlx[sel] ** 2
                                           + 0.5 * c[sel] * mly[sel] ** 2
                                           + b[sel] * mlx[sel] * mly[sel]
                                           - lnop[g[sel]])
            # delta colors: row k gets c_k - c_{k-1} in depth order across
            # blocks (c_{-1} = 0); dummy slots take the background color so
            # the first dummy row adds bg - c_{n-1} (the bg*T_final term)
            cseq = col[g]
            ext = np.zeros((bt * BLK, 3))
            ext[:n] = cseq
            ext[n:] = bg[None, :]
            dcs = np.diff(np.concatenate([np.zeros((1, 3)), ext], axis=0), axis=0)
            for bi in range(bt):
                s = gw * (goff[gg] + bi) + sub
                dc[d, :, s * 3:(s + 1) * 3] = dcs[bi * BLK:(bi + 1) * BLK]

    # pixel features in tile-local coords, replicated at the 3 PE quadrants
    xs = np.arange(TILE_C) + 0.5 - TILE_C / 2.0
    ys = np.arange(TILE_R) + 0.5 - TILE_R / 2.0
    Y, X = np.meshgrid(ys, xs, indexing="ij")
    x, y = X.ravel(), Y.ravel()
    feat6 = np.stack([x * x, y * y, x * y, x, y, np.ones(F)]).astype(np.float32)
    feat = np.zeros((128, F), np.float32)
    for qd in range(3):
        feat[32 * qd:32 * qd + 6] = feat6

    strict_u = np.triu(np.ones((BLK, BLK), np.float32), 1)
    compl_u = np.tril(np.ones((BLK, BLK), np.float32), 0)
    need_cu = any(b > 1 for b in gnblk)

    def trunc11(xv):
        # round-to-nearest at 11 explicit mantissa bits (the f32r PE input
        # precision); hi/lo stays exact and single-pass error is unbiased
        bb = np.ascontiguousarray(np.asarray(xv, np.float32)).view(np.uint32)
        return ((bb + np.uint32(0x800)) & np.uint32(0xFFFFF000)).view(np.float32)

    off_ut = 0
    off_cu = off_ut + BLK
    off_hi = off_cu + (BLK if need_cu else 0)
    off_lo = off_hi + nband * BLK
    off_dc = off_lo + nband * BLK
    off_ft = off_dc + nslot * 3
    XC = off_ft + F
    blobs = []
    for d in range(NDEV):
        blob = np.zeros((128, XC), np.float32)
        blob[:, off_ut:off_ut + BLK] = strict_u
        if need_cu:
            blob[:, off_cu:off_cu + BLK] = compl_u
        g32 = gt_slots[d].astype(np.float32)
        hi = trunc11(g32)
        lo = trunc11(g32 - hi)
        for s in range(nslot):
            p0, cb = 32 * (s % 3), (s // 3) * BLK
            blob[p0:p0 + 6, off_hi + cb:off_hi + cb + BLK] = hi[s]
            blob[p0:p0 + 6, off_lo + cb:off_lo + cb + BLK] = lo[s]
        blob[:, off_dc:off_dc + nslot * 3] = dc[d].astype(np.float32)
        blob[:, off_ft:off_ft + F] = feat
        blobs.append(blob)

    meta = dict(gw=gw, gnblk=gnblk, goff=list(map(int, goff)), nslot=nslot,
                nband=nband, need_cu=need_cu, XC=XC,
                offs=dict(ut=off_ut, cu=off_cu, hi=off_hi, lo=off_lo,
                          dc=off_dc, ft=off_ft),
                clamp_alpha=bool(np.asarray(opacities).max() >= ALPHA_MAX))
    return meta, blobs


def _patch_act_tables():
    """Resolve Exp and Ln to the combined table set so the compiler emits a
    single ACT table load instead of thrashing between per-func sets."""
    import functools
    import concourse.bacc as bacc_mod
    import concourse.mybir as mybir
    from concourse.hw_specs import get_activation_tables as orig

    if getattr(bacc_mod.get_activation_tables, "_combined_exp_ln", False):
        return

    @functools.cache
    def patched(arch):
        tabs = {k: set(v) for k, v in orig(arch).items()}
        combined = "natural_log_exp_and_others"
        if combined in tabs:
            Act = mybir.ActivationFunctionType
            for k in tabs:
                if k != combined:
                    tabs[k].discard(Act.Exp)
                    tabs[k].discard(Act.Ln)
        return tabs

    patched._combined_exp_ln = True
    bacc_mod.get_activation_tables = patched


def _build_program(meta, repeat=0, sb_bufs=4, work_bufs=None, col_bufs=None,
                   window=3, skew=None, split_dma=True, warmup_mms=10,
                   tail_mode=1, hilo=0, colp_own=None):
    import concourse.tile as tile
    import concourse.mybir as mybir
    from concourse import bacc
    from contextlib import ExitStack

    _patch_act_tables()
    f32 = mybir.dt.float32
    f32r = mybir.dt.float32r

    gw = meta["gw"]
    gnblk = meta["gnblk"]
    goff = meta["goff"]
    need_cu = meta["need_cu"]
    offs = meta["offs"]
    XC = meta["XC"]

    if colp_own is None:
        colp_own = gw >= 3 and not need_cu
    if work_bufs is None:
        if colp_own:
            work_bufs = 6 // gw
        elif need_cu:
            work_bufs = (8 - gw) // gw
        else:
            work_bufs = 8 // gw
    if col_bufs is None:
        col_bufs = 8 - work_bufs * gw if colp_own else 1

    nc = bacc.Bacc("TRN2", target_bir_lowering=False, debug=False)
    blob_d = nc.dram_tensor("blob", [128, XC], f32r, kind="ExternalInput")
    out_d = nc.dram_tensor("out", [3, STRIP, W], f32, kind="ExternalOutput")

    with tile.TileContext(nc) as tc, ExitStack() as ctx:
        cpool = ctx.enter_context(tc.tile_pool(name="consts", bufs=1))
        sb = ctx.enter_context(tc.tile_pool(name="sb", bufs=sb_bufs))
        work = ctx.enter_context(tc.tile_pool(name="work", bufs=work_bufs,
                                              space="PSUM"))
        colp_pool = None
        if need_cu or colp_own:
            colp_pool = ctx.enter_context(tc.tile_pool(name="colp", bufs=col_bufs,
                                                       space="PSUM"))

        if warmup_mms:
            # Ramp the PE pstate while the input DMA lands. Uses the work
            # pool's own rotation, so no extra PSUM banks.
            bf16 = mybir.dt.bfloat16
            wsrc = cpool.tile([BLK, 512], bf16, tag="warm_src")
            nc.gpsimd.memset(wsrc[:], 0)
            wdst = work.tile([BLK, 512 * gw], f32, tag="sig", name="warm")
            for _ in range(warmup_mms):
                nc.tensor.matmul(wdst[:, 0:512], wsrc[:, 0:BLK], wsrc[:],
                                 start=True, stop=True, skip_group_check=True)

        cst = cpool.tile([128, XC], f32r)
        if split_dma:
            nc.sync.dma_start(cst[0:64, :], blob_d.ap()[0:64, :])
            nc.gpsimd.dma_start(cst[64:128, :], blob_d.ap()[64:128, :])
        else:
            nc.sync.dma_start(cst[:], blob_d.ap())
        ut = cst[:, offs["ut"]:offs["ut"] + BLK]
        cu = cst[:, offs["cu"]:offs["cu"] + BLK] if need_cu else None
        ft = offs["ft"]

        def gt_ap(kind, s):
            p0, cb = 32 * (s % 3), (s // 3) * BLK
            base = offs[kind] + cb
            return cst[p0:p0 + 6, base:base + BLK]

        def feat_ap(s):
            p0 = 32 * (s % 3)
            return cst[p0:p0 + 6, ft:ft + F]

        def dc_ap(s):
            base = offs["dc"] + s * 3
            return cst[:, base:base + 3]

        out_ap = out_d.ap()

        def body():
            _emit(nc, tc, mybir, gw, gnblk, goff, need_cu, meta["clamp_alpha"],
                  gt_ap, feat_ap, dc_ap, ut, cu, sb, work, colp_pool,
                  out_ap, f32, f32r, window, skew, tail_mode=tail_mode,
                  hilo=hilo, colp_own=colp_own)

        if repeat:
            with tc.For_i(0, repeat, 1):
                body()
        else:
            body()
    nc.compile()
    return nc


def _emit(nc, tc, mybir, gw, gnblk, goff, need_cu, clamp_alpha,
          gt_ap, feat_ap, dc_ap, ut, cu, sb, work, colp_pool,
          out_ap, f32, f32r, window, skew, tail_mode=0, hilo=0, colp_own=False):
    Act = mybir.ActivationFunctionType
    Alu = mybir.AluOpType
    NGR = len(gnblk)
    FW = gw * F                # pixels per group op

    def gview(t):
        # [128, gw, 384] strided view over the gw used bank regions
        return t[:].rearrange("p (b c) -> p b c", b=gw)[:, :, 0:F]

    def make_stages(g):
        st = []
        state = {}
        bt = gnblk[g]

        for b in range(bt):
            last = b == bt - 1

            def s_sigma(b=b):
                sig = work.tile([BLK, 512 * gw], f32, tag="sig", name="sig")
                state["sig"] = sig
                for h in range(gw):
                    s = gw * (goff[g] + b) + h
                    o = 512 * h
                    if hilo:
                        nc.tensor.matmul(sig[:, o:o + F], gt_ap("hi", s),
                                         feat_ap(s), start=True, stop=False,
                                         skip_group_check=True)
                        nc.tensor.matmul(sig[:, o:o + F], gt_ap("lo", s),
                                         feat_ap(s), start=False, stop=True,
                                         skip_group_check=True)
                    else:
                        nc.tensor.matmul(sig[:, o:o + F], gt_ap("hi", s),
                                         feat_ap(s), start=True, stop=True,
                                         skip_group_check=True)

            def s_alpha():
                alpha = sb.tile([BLK, FW], f32, tag="alpha", name="alpha")
                state["alpha"] = alpha
                nc.scalar.activation(gview(alpha), gview(state["sig"]),
                                     Act.Exp, scale=-1.0)
                if clamp_alpha:
                    nc.vector.tensor_scalar_min(alpha[:], alpha[:], ALPHA_MAX)

            def s_ln():
                # unmasked ln(1-alpha): back-to-back with s_alpha on the ACT
                # queue (no DVE round-trip on the critical chain)
                lgr = sb.tile([BLK, FW], f32, tag="lgr", name="lgr")
                state["lgr"] = lgr
                nc.scalar.activation(lgr[:], state["alpha"][:], Act.Ln,
                                     bias=1.0, scale=-1.0)

            def s_am():
                # lg = (alpha >= 1/255) * ln(1-alpha), one DVE op; ln and
                # alpha stay back-to-back on the ACT queue
                lg = sb.tile([BLK, FW], f32r, tag="lg", name="lg")
                state["lg"] = lg
                nc.vector.scalar_tensor_tensor(lg[:], state["alpha"][:],
                                               ALPHA_MIN, state["lgr"][:],
                                               op0=Alu.is_ge, op1=Alu.mult)

            def s_strict(b=b, last=last):
                if b == 0:
                    # reuse the sigma PSUM tile: sig is dead after s_alpha and
                    # the first strict matmul start=True resets the banks
                    state["sS"] = state["sig"]
                sS = state["sS"]
                lg = state["lg"]
                for h in range(gw):
                    nc.tensor.matmul(sS[:, 512 * h:512 * h + F], ut,
                                     lg[:, F * h:F * (h + 1)],
                                     start=(b == 0), stop=(last and not need_cu),
                                     skip_group_check=True)

            def s_texp():
                tr = sb.tile([BLK, FW], f32r, tag="tr", name="tr")
                state["tr"] = tr
                nc.scalar.activation(gview(tr), gview(state["sS"]), Act.Exp)

            def s_color(b=b, last=last):
                if b == 0:
                    if colp_own:
                        # per-subtile single-bank accumulators from their own
                        # pool: the work tile frees right after texp, keeping
                        # the pipeline flowing with only 2 work bufs
                        state["colp"] = [colp_pool.tile([3, 512], f32,
                                                        tag="colp", name="colp")
                                         for _ in range(gw)]
                    elif need_cu:
                        cp = colp_pool.tile([3, 512 * gw], f32,
                                            tag="colp", name="colp")[:]
                        state["colp_full"] = cp
                        state["colp"] = [cp[:, 512 * h:512 * (h + 1)]
                                         for h in range(gw)]
                    else:
                        # single-block: the work tile is free after texp read
                        # it -> put the color accumulators there (WAR dep)
                        sp = state["sS"][0:3, :]
                        state["colp_full"] = sp
                        state["colp"] = [sp[:, 512 * h:512 * (h + 1)]
                                         for h in range(gw)]
                colp = state["colp"]
                tr = state["tr"]
                for h in range(gw):
                    s = gw * (goff[g] + b) + h
                    nc.tensor.matmul(colp[h][:, 0:F], dc_ap(s),
                                     tr[:, F * h:F * (h + 1)],
                                     start=(b == 0), stop=last,
                                     skip_group_check=True)
                if need_cu and not last:
                    sS, lg = state["sS"], state["lg"]
                    for h in range(gw):
                        nc.tensor.matmul(sS[:, 512 * h:512 * h + F], cu,
                                         lg[:, F * h:F * (h + 1)],
                                         start=False, stop=(b == bt - 2),
                                         skip_group_check=True)

            def s_out(g=g):
                colp = state["colp"]
                if (g == NGR - 1 and tail_mode) or colp_own:
                    # per-subtile copy/DMA chains: each colp bank drains as
                    # soon as its color matmul lands, and the last-iteration
                    # tail is one subtile deep instead of one group deep
                    for h in range(gw):
                        stg = sb.tile([3, F], f32, tag="stg", name="stg")
                        srcv = colp[h][:, 0:F]
                        if tail_mode == 2 and h % 2 == 1:
                            nc.scalar.copy(stg[:], srcv)
                        else:
                            nc.vector.tensor_copy(stg[:], srcv)
                        t = gw * g + h
                        dst = out_ap[:, :, t * TILE_C:(t + 1) * TILE_C]
                        src = stg[:].rearrange("c (h w) -> c h w", h=TILE_R)
                        if h % 2 == 0:
                            nc.gpsimd.dma_start(dst, src)
                        else:
                            nc.sync.dma_start(dst, src)
                    return
                stg = sb.tile([3, FW], f32, tag="stgw", name="stgw")
                # interleave the gw 24x16 subtiles into [3,24,gw*16] so the
                # out-DMA is a plain 3D transfer (DVE: gpsimd can't read PSUM)
                nc.vector.tensor_copy(
                    stg[:].rearrange("c (h b w) -> c b h w", b=gw, w=TILE_C),
                    state["colp_full"].rearrange("c (b x) -> c b x", b=gw)[:, :, 0:F]
                        .rearrange("c b (h w) -> c b h w", h=TILE_R))
                src = stg[:].rearrange("c (h w) -> c h w", h=TILE_R)
                dst = out_ap[:, :, gw * g * TILE_C:gw * (g + 1) * TILE_C]
                nc.gpsimd.dma_start(dst, src)

            st.extend([s_sigma, s_alpha, s_ln, s_am, s_strict, s_texp,
                       s_color])
            if last:
                st.append(s_out)
        return st

    stage_lists = [make_stages(g) for g in range(NGR)]

    if skew is not None:
        nst = [len(s) for s in stage_lists]
        total = skew * (NGR - 1) + max(nst)
        for step in range(total):
            for i, st in enumerate(stage_lists):
                s = step - skew * i
                if 0 <= s < len(st):
                    st[s]()
    else:
        i = 0
        while i < len(stage_lists):
            group = stage_lists[i:i + window]
            depth = max(len(s) for s in group)
            for s in range(depth):
                for gl in group:
                    if s < len(gl):
                        gl[s]()
            i += window


def _make_in_maps(blobs):
    return [{"blob": blobs[d]} for d in range(NDEV)]


def kernel(means2d, conics, colors, opacities, depths, background):
    from concourse import bass_utils

    meta, blobs = _host_prep(means2d, conics, colors, opacities, depths,
                             background)
    nc = _build_program(meta)
    in_maps = _make_in_maps(blobs)
    res = bass_utils.run_bass_kernel_spmd(nc, in_maps, core_ids=list(range(NDEV)))
    img = np.concatenate([res.results[d]["out"] for d in range(NDEV)], axis=1)
    return img.astype(np.float32)


if __name__ == "__main__":
    import reference

    inputs = {k: np.asarray(v) for k, v in reference.setup_inputs().items()}
    out = kernel(**inputs)
    print("kernel output:", out.shape, out.dtype)
